# revision 8
# baseline (speedup 1.0000x reference)
"""Trainium2 Bass kernel for batched 8-head local-window attention.

Shapes (hardcoded): x [32, 512, 512], w_qkv [512, 1536], w_proj [512, 512],
b_proj [512], mask [1, 1, 512, 512] additive (0 or -1e30).

Strategy: data-parallel over batch across 8 cores (4 batch elements each).
All matmuls in bf16 (fp32 PSUM accumulation). Layouts chosen so that no
input transposes are needed on device:
  - host supplies xT [C, N] per batch
  - qT,kT computed channel-major ([ch, n]) with w_qkv as stationary
  - v computed token-major ([n, ch]) with xT chunks as stationary
  - S^T = K @ Q^T per head ([m, n], key-major) so softmax sums arrive via a
    ones-column in the attn@V matmul; normalization is a per-partition scalar
  - attn@V uses masked exp(S^T) chunks as stationary, [v | 1] as moving
  - out head-concat is PE-transposed to channel-major for the projection
Mask is applied as a 0/1 multiply after exp (exp never sees -1e30; scores are
O(10) so no max-subtraction is needed for fp32/bf16 safety). Block-level
structure (which 128x128 chunks are entirely masked) is derived from the
actual mask argument at call time, so a dense (all-zero) mask also works.
"""

import numpy as np
import ml_dtypes

B, N, C = 32, 512, 512
HEADS = 8
HD = C // HEADS
SCALE = HD ** -0.5
NCORES = 8
BPC = B // NCORES  # batches per core
P = 128            # partitions
NT = N // P        # 4 n/m tiles of 128
CT = C // P        # 4 channel tiles of 128

_BF16 = ml_dtypes.bfloat16

_cache = {}


def _mask_structure(mask2d):
    """Derive block structure from the additive mask [n, m].

    Returns (W, offs, chunks) where offs[t] is the start column (query index)
    of the stored window for key-tile t, W the uniform window width, and
    chunks[s] the list of (t, lo, hi) key-tile chunks contributing to query
    block s (lo/hi = query index range covered, within [s*128, (s+1)*128)).
    """
    vis = mask2d == 0.0  # [n, m] True = visible
    assert vis.any(axis=1).all(), "some query attends to nothing"
    offs = []
    widths = []
    for t in range(NT):
        sub = vis[:, t * P:(t + 1) * P]  # [n, 128]
        rows = np.nonzero(sub.any(axis=1))[0]
        if len(rows) == 0:
            offs.append(0)
            widths.append(P)
            continue
        offs.append(int(rows.min()))
        widths.append(int(rows.max()) + 1 - int(rows.min()))
    W = max(widths)
    W = ((W + 63) // 64) * 64  # 64-align for clean APs
    W = min(W, N)
    offs = [min(o, N - W) for o in offs]
    chunks = []
    for s in range(NT):
        cl = []
        for t in range(NT):
            blk = vis[s * P:(s + 1) * P, t * P:(t + 1) * P]
            if not blk.any():
                continue
            lo = max(s * P, offs[t])
            hi = min((s + 1) * P, offs[t] + W)
            assert hi > lo
            # every visible query of this block must be inside [lo, hi)
            rows = np.nonzero(blk.any(axis=1))[0] + s * P
            assert rows.min() >= lo and rows.max() < hi
            cl.append((t, lo, hi))
        assert cl, f"query block {s} has no visible key chunks"
        # put a full-partition chunk first in the accumulation group (so the
        # start=True matmul initializes the whole PSUM partition range)
        cl.sort(key=lambda c: -(c[2] - c[1]))
        assert cl[0][2] - cl[0][1] == P
        chunks.append(cl)
    return W, offs, chunks


def _build(W, offs, chunks):
    import concourse.bass as bass
    import concourse.tile as tile
    import concourse.mybir as mybir
    from concourse import bacc
    from concourse.masks import make_identity

    fp32 = mybir.dt.float32
    bf16 = mybir.dt.bfloat16
    AF = mybir.ActivationFunctionType

    nc = bacc.Bacc("TRN2", target_bir_lowering=False, debug=False)

    d_xt = nc.dram_tensor("xt", [BPC, C, N], bf16, kind="ExternalInput")
    d_wqkv = nc.dram_tensor("wqkv", [C, 3 * C], bf16, kind="ExternalInput")
    d_wproj = nc.dram_tensor("wproj", [C, C], bf16, kind="ExternalInput")
    d_bvec = nc.dram_tensor("bvec", [1, C], bf16, kind="ExternalInput")
    d_m01 = nc.dram_tensor("m01", [P, NT, W], bf16, kind="ExternalInput")
    d_y = nc.dram_tensor("y", [BPC, N, C], fp32, kind="ExternalOutput")

    with tile.TileContext(nc) as tc:
        with (
            tc.tile_pool(name="singles", bufs=1) as singles,
            tc.tile_pool(name="xt", bufs=2) as xt_pool,
            tc.tile_pool(name="qk", bufs=2) as qk_pool,
            tc.tile_pool(name="vplus", bufs=2) as v_pool,
            tc.tile_pool(name="apair", bufs=2) as a_pool,
            tc.tile_pool(name="oc", bufs=2) as oc_pool,
            tc.tile_pool(name="rec", bufs=4) as rec_pool,
            tc.tile_pool(name="psS", bufs=2, space="PSUM") as psS_pool,
            tc.tile_pool(name="psB", bufs=2, space="PSUM") as psB_pool,
            tc.tile_pool(name="psT", bufs=1, space="PSUM") as psT_pool,
            tc.tile_pool(name="psO", bufs=1, space="PSUM") as psO_pool,
        ):
            # ---- persistent tiles ----
            wqkv = singles.tile([P, CT, 3 * C], bf16)
            nc.sync.dma_start(
                out=wqkv, in_=d_wqkv.ap().rearrange("(t p) o -> p t o", p=P))
            wproj = singles.tile([P, CT, C], bf16)
            nc.sync.dma_start(
                out=wproj, in_=d_wproj.ap().rearrange("(t p) o -> p t o", p=P))
            m01 = singles.tile([P, NT, W], bf16)
            nc.sync.dma_start(out=m01, in_=d_m01.ap())
            bvec = singles.tile([1, C], bf16)
            nc.sync.dma_start(out=bvec, in_=d_bvec.ap())
            ident = singles.tile([P, P], bf16)
            make_identity(nc, ident)
            ones_row = singles.tile([1, P], bf16)
            nc.vector.memset(ones_row, 1.0)

            for b in range(BPC):
                # ---- load xT (one DMA) ----
                xt = xt_pool.tile([P, CT, N], bf16)
                nc.sync.dma_start(
                    out=xt,
                    in_=d_xt.ap()[b].rearrange("(t p) n -> p t n", p=P))

                # ---- qT/kT: [ch-block, n] = w_chunk.T @ xT ----
                qk = qk_pool.tile([P, 2 * CT, N], bf16)
                for jj in range(2 * CT):
                    ps = psB_pool.tile([P, N], fp32, tag="psB")
                    for ct in range(CT):
                        nc.tensor.matmul(
                            ps,
                            lhsT=wqkv[:, ct, jj * P:(jj + 1) * P],
                            rhs=xt[:, ct, :],
                            start=(ct == 0), stop=(ct == CT - 1))
                    nc.vector.tensor_copy(out=qk[:, jj, :], in_=ps)

                # ---- v: [n-block, ch] = xT_chunk.T @ w_v ----
                vplus = v_pool.tile([P, NT, HEADS, HD + 1], bf16)
                for t in range(NT):
                    ps = psB_pool.tile([P, C], fp32, tag="psB")
                    for ct in range(CT):
                        nc.tensor.matmul(
                            ps,
                            lhsT=xt[:, ct, t * P:(t + 1) * P],
                            rhs=wqkv[:, ct, 2 * C:3 * C],
                            start=(ct == 0), stop=(ct == CT - 1))
                    nc.vector.tensor_copy(
                        out=vplus[:, t, :, 0:HD],
                        in_=ps.rearrange("p (h d) -> p h d", h=HEADS))
                nc.vector.memset(vplus[:, :, :, HD:HD + 1], 1.0)

                # ---- attention per head pair ----
                oc = oc_pool.tile([P, NT, C], bf16, tag="oc")
                for j in range(CT):  # heads 2j, 2j+1
                    apair = a_pool.tile([P, 2, NT, W], bf16)
                    for t in range(NT):
                        psp = psS_pool.tile([P, 2, N], fp32, tag="psS")
                        for hh in range(2):
                            sl = slice(hh * HD, (hh + 1) * HD)
                            nc.tensor.matmul(
                                psp[:, hh, 0:W],
                                lhsT=qk[sl, CT + j, t * P:(t + 1) * P],
                                rhs=qk[sl, j, offs[t]:offs[t] + W],
                                start=True, stop=True)
                        # exp for both heads of this key tile
                        nc.scalar.activation(
                            out=apair[:, :, t, :], in_=psp[:, :, 0:W],
                            func=AF.Exp)
                    # 0/1 mask multiply (split between DVE and GPSIMD)
                    eng = nc.vector if j < 1 else nc.gpsimd
                    for hh in range(2):
                        eng.tensor_mul(apair[:, hh], apair[:, hh], m01)

                    # ---- attn @ [v|1] per head, per query block ----
                    for hh in range(2):
                        h = 2 * j + hh
                        for s in range(NT):
                            pso = psO_pool.tile([P, HD + 1], fp32, tag="psO")
                            cl = chunks[s]
                            for ci, (t, lo, hi) in enumerate(cl):
                                nc.tensor.matmul(
                                    pso[lo - s * P:hi - s * P, :],
                                    lhsT=apair[:, hh, t,
                                               lo - offs[t]:hi - offs[t]],
                                    rhs=vplus[:, t, h, :],
                                    start=(ci == 0), stop=(ci == len(cl) - 1))
                            # normalize by the softmax denominator (col HD)
                            rec = rec_pool.tile([P, 1], fp32, tag="rec")
                            nc.vector.reciprocal(rec, pso[:, HD:HD + 1])
                            nc.vector.tensor_scalar_mul(
                                oc[:, s, h * HD:(h + 1) * HD],
                                pso[:, 0:HD], rec)

                # ---- transpose outcat to channel-major ----
                ocT = oc_pool.tile([P, CT, N], bf16, tag="ocT")
                for s in range(NT):
                    for ct in range(CT):
                        pst = psT_pool.tile([P, P], bf16, tag="psT")
                        nc.tensor.transpose(
                            pst, oc[:, s, ct * P:(ct + 1) * P], ident)
                        nc.vector.tensor_copy(
                            out=ocT[:, ct, s * P:(s + 1) * P], in_=pst)

                # ---- projection + bias; stream straight to DRAM ----
                for s in range(NT):
                    ps = psB_pool.tile([P, C], fp32, tag="psB")
                    for ct in range(CT):
                        nc.tensor.matmul(
                            ps,
                            lhsT=ocT[:, ct, s * P:(s + 1) * P],
                            rhs=wproj[:, ct, :],
                            start=(ct == 0), stop=False)
                    nc.tensor.matmul(
                        ps, lhsT=ones_row, rhs=bvec, start=False, stop=True)
                    ysb = rec_pool.tile([P, C], fp32, tag="ysb")
                    nc.vector.tensor_copy(out=ysb, in_=ps)
                    nc.sync.dma_start(
                        out=d_y.ap()[b, s * P:(s + 1) * P, :], in_=ysb)

    nc.compile()
    return nc


def _prep(x, w_qkv, w_proj, b_proj, mask):
    x = np.asarray(x, np.float32)
    w_qkv = np.asarray(w_qkv, np.float32)
    w_proj = np.asarray(w_proj, np.float32)
    b_proj = np.asarray(b_proj, np.float32)
    mask2d = np.asarray(mask, np.float32).reshape(N, N)

    W, offs, chunks = _mask_structure(mask2d)

    ws = w_qkv.copy()
    ws[:, :C] *= SCALE  # fold q scaling into the weights
    wqkv_b = ws.astype(_BF16)
    wproj_b = w_proj.astype(_BF16)
    bvec_b = b_proj.reshape(1, C).astype(_BF16)

    vis = (mask2d == 0.0)
    m01 = np.zeros((P, NT, W), np.float32)
    for t in range(NT):
        # m01[p, t, j] = visible(query=offs[t]+j, key=t*128+p)
        m01[:, t, :] = vis[offs[t]:offs[t] + W, t * P:(t + 1) * P].T
    m01_b = m01.astype(_BF16)

    # xT per core: [NCORES, BPC, C, N]
    xt = np.ascontiguousarray(
        x.reshape(NCORES, BPC, N, C).transpose(0, 1, 3, 2)).astype(_BF16)
    return xt, wqkv_b, wproj_b, bvec_b, m01_b, (W, tuple(offs),
                                               tuple(tuple(c) for c in chunks))


LAST_RESULTS = None


def kernel(x, w_qkv, w_proj, b_proj, mask, _trace=False):
    global LAST_RESULTS
    from concourse import bass_utils

    xt, wqkv_b, wproj_b, bvec_b, m01_b, key = _prep(
        x, w_qkv, w_proj, b_proj, mask)
    W, offs, chunks = key

    if key not in _cache:
        _cache[key] = _build(W, list(offs), [list(c) for c in chunks])
    nc = _cache[key]

    in_maps = []
    for core in range(NCORES):
        in_maps.append({
            "xt": xt[core],
            "wqkv": wqkv_b,
            "wproj": wproj_b,
            "bvec": bvec_b,
            "m01": m01_b,
        })
    res = bass_utils.run_bass_kernel_spmd(
        nc, in_maps, core_ids=list(range(NCORES)), trace=_trace)
    LAST_RESULTS = res
    y = np.concatenate([res.results[c]["y"] for c in range(NCORES)], axis=0)
    return y.reshape(B, N, C).astype(np.float32)


# revision 13
# speedup vs baseline: 1.0521x; 1.0521x over previous
"""Trainium2 Bass kernel for batched 8-head local-window attention.

Shapes (hardcoded): x [32, 512, 512], w_qkv [512, 1536], w_proj [512, 512],
b_proj [512], mask [1, 1, 512, 512] additive (0 or -1e30).

Strategy: data-parallel over batch across 8 cores (4 batch elements each).
All matmuls in bf16 (fp32 PSUM accumulation). Layouts chosen so that no
input transposes are needed on device:
  - host supplies xT [C, N] per batch
  - qT,kT computed channel-major ([ch, n]) with w_qkv as stationary
  - v computed token-major ([n, ch]) with xT chunks as stationary
  - S^T = K @ Q^T per head ([m, n], key-major) so softmax sums arrive via a
    ones-column in the attn@V matmul; normalization is a per-partition scalar
  - attn@V uses masked exp(S^T) chunks as stationary, [v | 1] as moving
  - out head-concat is PE-transposed to channel-major for the projection
Mask is applied as a 0/1 multiply after exp (exp never sees -1e30; scores are
O(10) so no max-subtraction is needed for fp32/bf16 safety). Block-level
structure (which 128x128 chunks are entirely masked) is derived from the
actual mask argument at call time, so a dense (all-zero) mask also works.
"""

import numpy as np
import ml_dtypes

B, N, C = 32, 512, 512
HEADS = 8
HD = C // HEADS
SCALE = HD ** -0.5
NCORES = 8
BPC = B // NCORES  # batches per core
P = 128            # partitions
NT = N // P        # 4 n/m tiles of 128
CT = C // P        # 4 channel tiles of 128

_BF16 = ml_dtypes.bfloat16

_cache = {}


def _mask_structure(mask2d):
    """Derive block structure from the additive mask [n, m].

    Returns (W, offs, chunks) where offs[t] is the start column (query index)
    of the stored window for key-tile t, W the uniform window width, and
    chunks[s] the list of (t, lo, hi) key-tile chunks contributing to query
    block s (lo/hi = query index range covered, within [s*128, (s+1)*128)).
    """
    vis = mask2d == 0.0  # [n, m] True = visible
    assert vis.any(axis=1).all(), "some query attends to nothing"
    offs = []
    widths = []
    for t in range(NT):
        sub = vis[:, t * P:(t + 1) * P]  # [n, 128]
        rows = np.nonzero(sub.any(axis=1))[0]
        if len(rows) == 0:
            offs.append(0)
            widths.append(P)
            continue
        offs.append(int(rows.min()))
        widths.append(int(rows.max()) + 1 - int(rows.min()))
    W = max(widths)
    W = ((W + 63) // 64) * 64  # 64-align for clean APs
    W = min(W, N)
    offs = [min(o, N - W) for o in offs]
    chunks = []
    for s in range(NT):
        cl = []
        for t in range(NT):
            blk = vis[s * P:(s + 1) * P, t * P:(t + 1) * P]
            if not blk.any():
                continue
            lo = max(s * P, offs[t])
            hi = min((s + 1) * P, offs[t] + W)
            assert hi > lo
            # every visible query of this block must be inside [lo, hi)
            rows = np.nonzero(blk.any(axis=1))[0] + s * P
            assert rows.min() >= lo and rows.max() < hi
            cl.append((t, lo, hi))
        assert cl, f"query block {s} has no visible key chunks"
        # put a full-partition chunk first in the accumulation group (so the
        # start=True matmul initializes the whole PSUM partition range)
        cl.sort(key=lambda c: -(c[2] - c[1]))
        assert cl[0][2] - cl[0][1] == P
        chunks.append(cl)
    return W, offs, chunks


def _build(W, offs, chunks):
    import concourse.bass as bass
    import concourse.tile as tile
    import concourse.mybir as mybir
    from concourse import bacc
    from concourse.masks import make_identity

    fp32 = mybir.dt.float32
    bf16 = mybir.dt.bfloat16
    AF = mybir.ActivationFunctionType

    nc = bacc.Bacc("TRN2", target_bir_lowering=False, debug=False)

    d_xt = nc.dram_tensor("xt", [BPC, C, N], bf16, kind="ExternalInput")
    d_wqkv = nc.dram_tensor("wqkv", [C, 3 * C], bf16, kind="ExternalInput")
    d_wproj = nc.dram_tensor("wproj", [C, C], bf16, kind="ExternalInput")
    d_bvec = nc.dram_tensor("bvec", [1, C], bf16, kind="ExternalInput")
    d_m01 = nc.dram_tensor("m01", [P, NT, W], bf16, kind="ExternalInput")
    d_y = nc.dram_tensor("y", [BPC, N, C], fp32, kind="ExternalOutput")

    with tile.TileContext(nc) as tc:
        with (
            tc.tile_pool(name="singles", bufs=1) as singles,
            tc.tile_pool(name="xt", bufs=2) as xt_pool,
            tc.tile_pool(name="qk", bufs=2) as qk_pool,
            tc.tile_pool(name="vplus", bufs=2) as v_pool,
            tc.tile_pool(name="apair", bufs=5) as a_pool,
            tc.tile_pool(name="oc", bufs=2) as oc_pool,
            tc.tile_pool(name="rec", bufs=4) as rec_pool,
            tc.tile_pool(name="psS", bufs=2, space="PSUM") as psS_pool,
            tc.tile_pool(name="psB", bufs=2, space="PSUM") as psB_pool,
            tc.tile_pool(name="psO", bufs=1, space="PSUM") as psO_pool,
        ):
            # ---- persistent tiles ----
            wqkv = singles.tile([P, CT, 3 * C], bf16)
            nc.sync.dma_start(
                out=wqkv, in_=d_wqkv.ap().rearrange("(t p) o -> p t o", p=P))
            wproj = singles.tile([P, CT, C], bf16)
            nc.sync.dma_start(
                out=wproj, in_=d_wproj.ap().rearrange("(t p) o -> p t o", p=P))
            m01 = singles.tile([P, NT, W], bf16)
            nc.sync.dma_start(out=m01, in_=d_m01.ap())
            bvec = singles.tile([1, C], bf16)
            nc.sync.dma_start(out=bvec, in_=d_bvec.ap())
            ident = singles.tile([P, P], bf16)
            make_identity(nc, ident)
            ones_row = singles.tile([1, P], bf16)
            nc.vector.memset(ones_row, 1.0)

            for b in range(BPC):
                # ---- load xT (one DMA) ----
                xt = xt_pool.tile([P, CT, N], bf16)
                nc.sync.dma_start(
                    out=xt,
                    in_=d_xt.ap()[b].rearrange("(t p) n -> p t n", p=P))

                # ---- qT/kT: [ch-block, n] = w_chunk.T @ xT ----
                qk = qk_pool.tile([P, 2 * CT, N], bf16)
                for jj in range(2 * CT):
                    ps = psB_pool.tile([P, N], fp32, tag="psB")
                    for ct in range(CT):
                        nc.tensor.matmul(
                            ps,
                            lhsT=wqkv[:, ct, jj * P:(jj + 1) * P],
                            rhs=xt[:, ct, :],
                            start=(ct == 0), stop=(ct == CT - 1))
                    if jj % 2 == 0:
                        nc.vector.tensor_copy(out=qk[:, jj, :], in_=ps)
                    else:
                        nc.scalar.copy(out=qk[:, jj, :], in_=ps)

                # ---- v: [n-block, ch] = xT_chunk.T @ w_v ----
                vplus = v_pool.tile([P, NT, HEADS, HD + 1], bf16)
                for t in range(NT):
                    ps = psB_pool.tile([P, C], fp32, tag="psB")
                    for ct in range(CT):
                        nc.tensor.matmul(
                            ps,
                            lhsT=xt[:, ct, t * P:(t + 1) * P],
                            rhs=wqkv[:, ct, 2 * C:3 * C],
                            start=(ct == 0), stop=(ct == CT - 1))
                    nc.vector.tensor_copy(
                        out=vplus[:, t, :, 0:HD],
                        in_=ps.rearrange("p (h d) -> p h d", h=HEADS))
                nc.vector.memset(vplus[:, :, :, HD:HD + 1], 1.0)

                # ---- attention: scores + exp + mask, per head pair ----
                oc = oc_pool.tile([P, NT, C], bf16, tag="oc")
                apairs = []
                for j in range(CT):  # heads 2j, 2j+1
                    apair = a_pool.tile([P, 2, NT, W], bf16)
                    apairs.append(apair)
                    for t in range(NT):
                        psp = psS_pool.tile([P, 2, N], fp32, tag="psS")
                        for hh in range(2):
                            sl = slice(hh * HD, (hh + 1) * HD)
                            nc.tensor.matmul(
                                psp[:, hh, 0:W],
                                lhsT=qk[sl, CT + j, t * P:(t + 1) * P],
                                rhs=qk[sl, j, offs[t]:offs[t] + W],
                                start=True, stop=True)
                        # exp for both heads of this key tile
                        nc.scalar.activation(
                            out=apair[:, :, t, :], in_=psp[:, :, 0:W],
                            func=AF.Exp)
                    # 0/1 mask multiply (split between DVE and GPSIMD)
                    for hh in range(2):
                        eng = nc.vector if 2 * j + hh < 5 else nc.gpsimd
                        eng.tensor_mul(apair[:, hh], apair[:, hh], m01)

                # ---- attn @ [v|1]: all 8 heads per query block ----
                # pso spans 2 PSUM banks; heads 0..3 in bank 0 (start=True on
                # head 0 only), heads 4..7 in bank 1 (start=True on head 4).
                # Later heads rely on the bank-wide pending-zero from start.
                for s in range(NT):
                    pso = psO_pool.tile([P, HEADS, P], fp32, tag="psO")
                    cl = chunks[s]
                    for h in range(HEADS):
                        for ci, (t, lo, hi) in enumerate(cl):
                            nc.tensor.matmul(
                                pso[lo - s * P:hi - s * P, h, 0:HD + 1],
                                lhsT=apairs[h // 2][:, h % 2, t,
                                                    lo - offs[t]:hi - offs[t]],
                                rhs=vplus[:, t, h, :],
                                start=(h % 4 == 0 and ci == 0),
                                stop=(h % 4 == 3 and ci == len(cl) - 1),
                                skip_group_check=True)
                    # normalize all heads at once: oc = pso[:, :, :64] * rec
                    rec = rec_pool.tile([P, HEADS], fp32, tag="rec")
                    nc.vector.reciprocal(rec, pso[:, :, HD])
                    ra = rec[:, :]
                    rec_b = bass.AP(
                        tensor=ra.tensor, offset=ra.offset,
                        ap=[ra.ap[0], [1, HEADS], [0, HD]])
                    nc.vector.tensor_mul(
                        oc[:, s, :].rearrange("p (h d) -> p h d", h=HEADS),
                        pso[:, :, 0:HD], rec_b)

                # ---- transpose outcat to channel-major ----
                ocT = oc_pool.tile([P, CT, N], bf16, tag="ocT")
                for ct in range(CT):
                    pst = psB_pool.tile([P, N], bf16, tag="psB")
                    for s in range(NT):
                        nc.tensor.matmul(
                            pst[:, s * P:(s + 1) * P],
                            lhsT=oc[:, s, ct * P:(ct + 1) * P],
                            rhs=ident, is_transpose=True,
                            start=(s == 0), stop=(s == NT - 1),
                            skip_group_check=True)
                    nc.vector.tensor_copy(out=ocT[:, ct, :], in_=pst)

                # ---- projection + bias; stream straight to DRAM ----
                for s in range(NT):
                    ps = psB_pool.tile([P, C], fp32, tag="psB")
                    for ct in range(CT):
                        nc.tensor.matmul(
                            ps,
                            lhsT=ocT[:, ct, s * P:(s + 1) * P],
                            rhs=wproj[:, ct, :],
                            start=(ct == 0), stop=False)
                    nc.tensor.matmul(
                        ps, lhsT=ones_row, rhs=bvec, start=False, stop=True)
                    ysb = rec_pool.tile([P, C], fp32, tag="ysb")
                    nc.vector.tensor_copy(out=ysb, in_=ps)
                    nc.sync.dma_start(
                        out=d_y.ap()[b, s * P:(s + 1) * P, :], in_=ysb)

    nc.compile()
    return nc


def _prep(x, w_qkv, w_proj, b_proj, mask):
    x = np.asarray(x, np.float32)
    w_qkv = np.asarray(w_qkv, np.float32)
    w_proj = np.asarray(w_proj, np.float32)
    b_proj = np.asarray(b_proj, np.float32)
    mask2d = np.asarray(mask, np.float32).reshape(N, N)

    W, offs, chunks = _mask_structure(mask2d)

    ws = w_qkv.copy()
    ws[:, :C] *= SCALE  # fold q scaling into the weights
    wqkv_b = ws.astype(_BF16)
    wproj_b = w_proj.astype(_BF16)
    bvec_b = b_proj.reshape(1, C).astype(_BF16)

    vis = (mask2d == 0.0)
    m01 = np.zeros((P, NT, W), np.float32)
    for t in range(NT):
        # m01[p, t, j] = visible(query=offs[t]+j, key=t*128+p)
        m01[:, t, :] = vis[offs[t]:offs[t] + W, t * P:(t + 1) * P].T
    m01_b = m01.astype(_BF16)

    # xT per core: [NCORES, BPC, C, N]
    xt = np.ascontiguousarray(
        x.reshape(NCORES, BPC, N, C).transpose(0, 1, 3, 2)).astype(_BF16)
    return xt, wqkv_b, wproj_b, bvec_b, m01_b, (W, tuple(offs),
                                               tuple(tuple(c) for c in chunks))


LAST_RESULTS = None


def kernel(x, w_qkv, w_proj, b_proj, mask, _trace=False):
    global LAST_RESULTS
    from concourse import bass_utils

    xt, wqkv_b, wproj_b, bvec_b, m01_b, key = _prep(
        x, w_qkv, w_proj, b_proj, mask)
    W, offs, chunks = key

    if key not in _cache:
        _cache[key] = _build(W, list(offs), [list(c) for c in chunks])
    nc = _cache[key]

    in_maps = []
    for core in range(NCORES):
        in_maps.append({
            "xt": xt[core],
            "wqkv": wqkv_b,
            "wproj": wproj_b,
            "bvec": bvec_b,
            "m01": m01_b,
        })
    res = bass_utils.run_bass_kernel_spmd(
        nc, in_maps, core_ids=list(range(NCORES)), trace=_trace)
    LAST_RESULTS = res
    y = np.concatenate([res.results[c]["y"] for c in range(NCORES)], axis=0)
    return y.reshape(B, N, C).astype(np.float32)


# revision 14
# speedup vs baseline: 1.3451x; 1.2785x over previous
"""Trainium2 Bass kernel for batched 8-head local-window attention.

Shapes (hardcoded): x [32, 512, 512], w_qkv [512, 1536], w_proj [512, 512],
b_proj [512], mask [1, 1, 512, 512] additive (0 or -1e30).

Strategy: data-parallel over batch across 8 cores (4 batch elements each).
All matmuls in bf16 (fp32 PSUM accumulation). Layouts chosen so that no
input transposes are needed on device:
  - host supplies xT [C, N] per batch
  - qT,kT computed channel-major ([ch, n]) with w_qkv as stationary
  - v computed token-major ([n, ch]) with xT chunks as stationary
  - S^T = K @ Q^T per head ([m, n], key-major) so softmax sums arrive via a
    ones-column in the attn@V matmul; normalization is a per-partition scalar
  - attn@V uses masked exp(S^T) chunks as stationary, [v | 1] as moving
  - out head-concat is PE-transposed to channel-major for the projection
Mask is applied as a 0/1 multiply after exp (exp never sees -1e30; scores are
O(10) so no max-subtraction is needed for fp32/bf16 safety). Block-level
structure (which 128x128 chunks are entirely masked) is derived from the
actual mask argument at call time, so a dense (all-zero) mask also works.
"""

import numpy as np
import ml_dtypes

B, N, C = 32, 512, 512
HEADS = 8
HD = C // HEADS
SCALE = HD ** -0.5
NCORES = 8
BPC = B // NCORES  # batches per core
P = 128            # partitions
NT = N // P        # 4 n/m tiles of 128
CT = C // P        # 4 channel tiles of 128

_BF16 = ml_dtypes.bfloat16

_cache = {}


def _mask_structure(mask2d):
    """Derive block structure from the additive mask [n, m].

    Returns (W, offs, chunks) where offs[t] is the start column (query index)
    of the stored window for key-tile t, W the uniform window width, and
    chunks[s] the list of (t, lo, hi) key-tile chunks contributing to query
    block s (lo/hi = query index range covered, within [s*128, (s+1)*128)).
    """
    vis = mask2d == 0.0  # [n, m] True = visible
    assert vis.any(axis=1).all(), "some query attends to nothing"
    offs = []
    widths = []
    for t in range(NT):
        sub = vis[:, t * P:(t + 1) * P]  # [n, 128]
        rows = np.nonzero(sub.any(axis=1))[0]
        if len(rows) == 0:
            offs.append(0)
            widths.append(P)
            continue
        offs.append(int(rows.min()))
        widths.append(int(rows.max()) + 1 - int(rows.min()))
    W = max(widths)
    W = ((W + 63) // 64) * 64  # 64-align for clean APs
    W = min(W, N)
    offs = [min(o, N - W) for o in offs]
    chunks = []
    for s in range(NT):
        cl = []
        for t in range(NT):
            blk = vis[s * P:(s + 1) * P, t * P:(t + 1) * P]
            if not blk.any():
                continue
            lo = max(s * P, offs[t])
            hi = min((s + 1) * P, offs[t] + W)
            assert hi > lo
            # every visible query of this block must be inside [lo, hi)
            rows = np.nonzero(blk.any(axis=1))[0] + s * P
            assert rows.min() >= lo and rows.max() < hi
            cl.append((t, lo, hi))
        assert cl, f"query block {s} has no visible key chunks"
        # put a full-partition chunk first in the accumulation group (so the
        # start=True matmul initializes the whole PSUM partition range)
        cl.sort(key=lambda c: -(c[2] - c[1]))
        assert cl[0][2] - cl[0][1] == P
        chunks.append(cl)
    return W, offs, chunks


def _build(W, offs, chunks):
    import concourse.bass as bass
    import concourse.tile as tile
    import concourse.mybir as mybir
    from concourse import bacc
    from concourse.masks import make_identity

    fp32 = mybir.dt.float32
    bf16 = mybir.dt.bfloat16
    AF = mybir.ActivationFunctionType

    nc = bacc.Bacc("TRN2", target_bir_lowering=False, debug=False)

    d_xt = nc.dram_tensor("xt", [BPC, C, N], bf16, kind="ExternalInput")
    d_wqkv = nc.dram_tensor("wqkv", [C, 3 * C], bf16, kind="ExternalInput")
    d_wproj = nc.dram_tensor("wproj", [C, C], bf16, kind="ExternalInput")
    d_bvec = nc.dram_tensor("bvec", [1, C], bf16, kind="ExternalInput")
    d_m01 = nc.dram_tensor("m01", [P, NT, W], bf16, kind="ExternalInput")
    d_y = nc.dram_tensor("y", [BPC, N, C], fp32, kind="ExternalOutput")

    with tile.TileContext(nc) as tc:
        with (
            tc.tile_pool(name="singles", bufs=1) as singles,
            tc.tile_pool(name="xt", bufs=2) as xt_pool,
            tc.tile_pool(name="qk", bufs=2) as qk_pool,
            tc.tile_pool(name="vplus", bufs=2) as v_pool,
            tc.tile_pool(name="apair", bufs=5) as a_pool,
            tc.tile_pool(name="oc", bufs=2) as oc_pool,
            tc.tile_pool(name="rec", bufs=4) as rec_pool,
            tc.tile_pool(name="psS", bufs=2, space="PSUM") as psS_pool,
            tc.tile_pool(name="psB", bufs=2, space="PSUM") as psB_pool,
            tc.tile_pool(name="psO", bufs=1, space="PSUM") as psO_pool,
        ):
            # ---- persistent tiles ----
            wqkv = singles.tile([P, CT, 3 * C], bf16)
            nc.sync.dma_start(
                out=wqkv, in_=d_wqkv.ap().rearrange("(t p) o -> p t o", p=P))
            wproj = singles.tile([P, CT, C], bf16)
            nc.sync.dma_start(
                out=wproj, in_=d_wproj.ap().rearrange("(t p) o -> p t o", p=P))
            m01 = singles.tile([P, NT, W], bf16)
            nc.sync.dma_start(out=m01, in_=d_m01.ap())
            bvec = singles.tile([1, C], bf16)
            nc.sync.dma_start(out=bvec, in_=d_bvec.ap())
            ident = singles.tile([P, P], bf16)
            make_identity(nc, ident)
            ones_row = singles.tile([1, P], bf16)
            nc.vector.memset(ones_row, 1.0)

            def qkv_phase(b):
                """Load xT, compute qT/kT (channel-major) and v (token-major).

                Returns (qk, vplus) tiles for the batch.
                """
                xt = xt_pool.tile([P, CT, N], bf16, tag="xt")
                nc.sync.dma_start(
                    out=xt,
                    in_=d_xt.ap()[b].rearrange("(t p) n -> p t n", p=P))
                qk = qk_pool.tile([P, 2 * CT, N], bf16, tag="qk")
                for jj in range(2 * CT):
                    ps = psB_pool.tile([P, N], fp32, tag="psB")
                    for ct in range(CT):
                        nc.tensor.matmul(
                            ps,
                            lhsT=wqkv[:, ct, jj * P:(jj + 1) * P],
                            rhs=xt[:, ct, :],
                            start=(ct == 0), stop=(ct == CT - 1))
                    if jj % 2 == 0:
                        nc.vector.tensor_copy(out=qk[:, jj, :], in_=ps)
                    else:
                        nc.scalar.copy(out=qk[:, jj, :], in_=ps)
                vplus = v_pool.tile([P, NT, HEADS, HD + 1], bf16, tag="vplus")
                for t in range(NT):
                    ps = psB_pool.tile([P, C], fp32, tag="psB")
                    for ct in range(CT):
                        nc.tensor.matmul(
                            ps,
                            lhsT=xt[:, ct, t * P:(t + 1) * P],
                            rhs=wqkv[:, ct, 2 * C:3 * C],
                            start=(ct == 0), stop=(ct == CT - 1))
                    nc.vector.tensor_copy(
                        out=vplus[:, t, :, 0:HD],
                        in_=ps.rearrange("p (h d) -> p h d", h=HEADS))
                nc.vector.memset(vplus[:, :, :, HD:HD + 1], 1.0)
                return qk, vplus

            def score_phase(qk):
                """S^T = K@Q^T (row-packed head pairs), exp, 0/1 mask mul."""
                apairs = []
                for j in range(CT):  # heads 2j, 2j+1
                    apair = a_pool.tile([P, 2, NT, W], bf16, tag="apair")
                    apairs.append(apair)
                    for t in range(NT):
                        psp = psS_pool.tile([P, 2, N], fp32, tag="psS")
                        for hh in range(2):
                            sl = slice(hh * HD, (hh + 1) * HD)
                            nc.tensor.matmul(
                                psp[:, hh, 0:W],
                                lhsT=qk[sl, CT + j, t * P:(t + 1) * P],
                                rhs=qk[sl, j, offs[t]:offs[t] + W],
                                start=True, stop=True)
                        nc.scalar.activation(
                            out=apair[:, :, t, :], in_=psp[:, :, 0:W],
                            func=AF.Exp)
                    for hh in range(2):
                        eng = nc.vector if 2 * j + hh < 6 else nc.gpsimd
                        eng.tensor_mul(apair[:, hh], apair[:, hh], m01)
                return apairs

            def attnv_phase(apairs, vplus):
                """attn @ [v|1] for 4 heads at a time per query block; each
                pso tile is one PSUM bank — start=True only on the first
                matmul of the bank, later heads use the pending-zero."""
                oc = oc_pool.tile([P, NT, C], bf16, tag="oc")
                for s in range(NT):
                    cl = chunks[s]
                    for g in range(2):  # head groups 0-3, 4-7
                        pso = psO_pool.tile([P, 4, P], fp32, tag="psO")
                        for hh in range(4):
                            h = 4 * g + hh
                            for ci, (t, lo, hi) in enumerate(cl):
                                nc.tensor.matmul(
                                    pso[lo - s * P:hi - s * P, hh, 0:HD + 1],
                                    lhsT=apairs[h // 2][
                                        :, h % 2, t, lo - offs[t]:hi - offs[t]],
                                    rhs=vplus[:, t, h, :],
                                    start=(hh == 0 and ci == 0),
                                    stop=(hh == 3 and ci == len(cl) - 1),
                                    skip_group_check=True)
                        rec = rec_pool.tile([P, 4], fp32, tag="rec")
                        nc.vector.reciprocal(rec, pso[:, :, HD])
                        ra = rec[:, :]
                        rec_b = bass.AP(
                            tensor=ra.tensor, offset=ra.offset,
                            ap=[ra.ap[0], [1, 4], [0, HD]])
                        nc.vector.tensor_mul(
                            oc[:, s, g * C // 2:(g + 1) * C // 2].rearrange(
                                "p (h d) -> p h d", h=4),
                            pso[:, :, 0:HD], rec_b)
                return oc

            def out_phase(oc, b):
                """Transpose outcat to channel-major, project, add bias."""
                ocT = oc_pool.tile([P, CT, N], bf16, tag="ocT")
                for ct in range(CT):
                    pst = psB_pool.tile([P, N], bf16, tag="psB")
                    for s in range(NT):
                        nc.tensor.matmul(
                            pst[:, s * P:(s + 1) * P],
                            lhsT=oc[:, s, ct * P:(ct + 1) * P],
                            rhs=ident, is_transpose=True,
                            start=(s == 0), stop=(s == NT - 1),
                            skip_group_check=True)
                    nc.vector.tensor_copy(out=ocT[:, ct, :], in_=pst)
                for s in range(NT):
                    ps = psB_pool.tile([P, C], fp32, tag="psB")
                    for ct in range(CT):
                        nc.tensor.matmul(
                            ps,
                            lhsT=ocT[:, ct, s * P:(s + 1) * P],
                            rhs=wproj[:, ct, :],
                            start=(ct == 0), stop=False)
                    nc.tensor.matmul(
                        ps, lhsT=ones_row, rhs=bvec, start=False, stop=True)
                    ysb = rec_pool.tile([P, C], fp32, tag="ysb")
                    nc.vector.tensor_copy(out=ysb, in_=ps)
                    nc.sync.dma_start(
                        out=d_y.ap()[b, s * P:(s + 1) * P, :], in_=ysb)

            # Software-pipelined batch loop: batch b+1's qkv matmuls are
            # emitted between batch b's score phase and attnV phase, so the
            # PE has dense work while ACT/DVE run exp + mask (keeps HAM warm).
            qk, vplus = qkv_phase(0)
            for b in range(BPC):
                apairs = score_phase(qk)
                if b + 1 < BPC:
                    qk_n, vplus_n = qkv_phase(b + 1)
                else:
                    qk_n = vplus_n = None
                oc = attnv_phase(apairs, vplus)
                out_phase(oc, b)
                qk, vplus = qk_n, vplus_n

    nc.compile()
    return nc


def _prep(x, w_qkv, w_proj, b_proj, mask):
    x = np.asarray(x, np.float32)
    w_qkv = np.asarray(w_qkv, np.float32)
    w_proj = np.asarray(w_proj, np.float32)
    b_proj = np.asarray(b_proj, np.float32)
    mask2d = np.asarray(mask, np.float32).reshape(N, N)

    W, offs, chunks = _mask_structure(mask2d)

    ws = w_qkv.copy()
    ws[:, :C] *= SCALE  # fold q scaling into the weights
    wqkv_b = ws.astype(_BF16)
    wproj_b = w_proj.astype(_BF16)
    bvec_b = b_proj.reshape(1, C).astype(_BF16)

    vis = (mask2d == 0.0)
    m01 = np.zeros((P, NT, W), np.float32)
    for t in range(NT):
        # m01[p, t, j] = visible(query=offs[t]+j, key=t*128+p)
        m01[:, t, :] = vis[offs[t]:offs[t] + W, t * P:(t + 1) * P].T
    m01_b = m01.astype(_BF16)

    # xT per core: [NCORES, BPC, C, N]
    xt = np.ascontiguousarray(
        x.reshape(NCORES, BPC, N, C).transpose(0, 1, 3, 2)).astype(_BF16)
    return xt, wqkv_b, wproj_b, bvec_b, m01_b, (W, tuple(offs),
                                               tuple(tuple(c) for c in chunks))


LAST_RESULTS = None


def kernel(x, w_qkv, w_proj, b_proj, mask, _trace=False):
    global LAST_RESULTS
    from concourse import bass_utils

    xt, wqkv_b, wproj_b, bvec_b, m01_b, key = _prep(
        x, w_qkv, w_proj, b_proj, mask)
    W, offs, chunks = key

    if key not in _cache:
        _cache[key] = _build(W, list(offs), [list(c) for c in chunks])
    nc = _cache[key]

    in_maps = []
    for core in range(NCORES):
        in_maps.append({
            "xt": xt[core],
            "wqkv": wqkv_b,
            "wproj": wproj_b,
            "bvec": bvec_b,
            "m01": m01_b,
        })
    res = bass_utils.run_bass_kernel_spmd(
        nc, in_maps, core_ids=list(range(NCORES)), trace=_trace)
    LAST_RESULTS = res
    y = np.concatenate([res.results[c]["y"] for c in range(NCORES)], axis=0)
    return y.reshape(B, N, C).astype(np.float32)
